# revision 2
# baseline (speedup 1.0000x reference)
"""Conv2DMod (StyleGAN2-style modulated conv) Trainium2 Bass kernel.

Problem: B=8, C_in=512, C_out=512, K=3x3, H=W=64, fp32, 'same' padding.

  wts[b,o,c,kh,kw] = weight[o,c,kh,kw] * (y[b,c]+1)
  d[b,o]           = rsqrt(sum_{c,kh,kw} wts^2 + 1e-8)
  out[b]           = conv2d(x[b], wts[b]*d[b,o])

Strategy (data-parallel over batch, one sample per NeuronCore, 8 cores):
  conv is linear in the weight; the modulation scale s_c=(y+1) depends
  only on the input channel and the demod d_o only on the output
  channel, so
      out = d_o * conv(s_c * x, weight)
  Each core:
    1. preloads ALL weights as bf16 tiles (host pre-transposed to
       [k, c, o]) and the full image, scaled by s_c into 4 padded bf16
       c-tiles held in SBUF,
    2. runs the 3x3 conv as 9 shift-matmuls per c-tile, accumulating
       ALL 36 matmuls (9 taps x 4 c-tiles) of an output chunk in a
       single PSUM bank group — one eviction per chunk,
    3. computes d_o on the PE (sum_c s2[c]*w2[c,o] via M=1 matmuls of
       squared weight tiles) and applies it on the PSUM->SBUF eviction.

bf16 operands keep the PE at full streaming rate (same as fp32r on
TRN2), halve weight DMA, and need no on-device fp32r conversion pass;
PSUM accumulation stays fp32 so the rel err is ~3e-3.

kernel(x, y, weight) takes the FULL unsharded inputs and returns the
full (8, 512, 64, 64) fp32 output.
"""

import numpy as np

import concourse.bass as bass
import concourse.tile as tile
from concourse import bacc, mybir
from concourse.bass_utils import run_bass_kernel_spmd

# Problem constants (hardcoded per spec).
B = 8
C = 512          # input channels
O = 512          # output channels
H = W = 64
KK = 9           # 3x3 taps
PR = PW = 66     # padded image rows/cols
CT = 4           # c tiles of 128
OT = 4           # o tiles of 128
NCH = 8          # hw chunks: 8 rows x 64 cols = 512 free elems
ROWS = 8
EPS = 1e-8

F32 = mybir.dt.float32
F32R = mybir.dt.float32r
BF16 = mybir.dt.bfloat16
AF = mybir.ActivationFunctionType


def build_nc(reps=1):
    nc = bacc.Bacc(None, target_bir_lowering=False)

    x_d = nc.dram_tensor("x", [C, H, W], F32, kind="ExternalInput")
    y_d = nc.dram_tensor("y", [1, C], F32, kind="ExternalInput")
    wt_d = nc.dram_tensor("wt", [KK, C, O], BF16, kind="ExternalInput")
    out_d = nc.dram_tensor("out", [O, H, W], F32, kind="ExternalOutput")

    with tile.TileContext(nc) as tc:
      for _rep in range(reps):
        with (
            tc.tile_pool(name="wsb", bufs=1) as w_pool,
            tc.tile_pool(name="xpad", bufs=1) as xpad_pool,
            tc.tile_pool(name="xstg", bufs=4) as xstg_pool,
            tc.tile_pool(name="wt2", bufs=3) as wt2_pool,
            tc.tile_pool(name="osb", bufs=4) as osb_pool,
            tc.tile_pool(name="small", bufs=1) as small_pool,
            tc.tile_pool(name="cpsum", bufs=6, space=bass.MemorySpace.PSUM) as cpsum_pool,
            tc.tile_pool(name="vpsum", bufs=1, space=bass.MemorySpace.PSUM) as vpsum_pool,
        ):
            # ---- small setup: s = y+1, transposed to per-partition cols
            y_sb = small_pool.tile([1, C], F32, tag="y")
            nc.sync.dma_start(y_sb[:], y_d[:])

            ones = small_pool.tile([1, 1], F32, tag="ones")
            nc.vector.memset(ones[:], 1.0)
            eps_1 = small_pool.tile([1, 1], F32, tag="eps1")
            nc.vector.memset(eps_1[:], EPS)
            zero_col = small_pool.tile([128, 1], F32, tag="zerocol")
            nc.vector.memset(zero_col[:], 0.0)

            s_row = small_pool.tile([1, C], F32, tag="srow")
            nc.scalar.activation(s_row[:], y_sb[:], AF.Identity,
                                 bias=ones[0:1, 0:1])

            # K=1 matmul transposes a row-vector slice into a psum column.
            s_col = small_pool.tile([128, CT], F32, tag="scol")
            for t in range(CT):
                ps = vpsum_pool.tile([128, 1], F32, name="ps")
                nc.tensor.matmul(
                    ps[:], s_row[0:1, t * 128:(t + 1) * 128], ones[0:1, 0:1],
                    start=True, stop=True,
                )
                nc.scalar.copy(s_col[:, t:t + 1], ps[:])
            # fp32r matmul operands must be engine-produced with F32R
            # output dtype.
            s2_col = small_pool.tile([128, CT], F32R, tag="s2col")
            nc.vector.tensor_mul(s2_col[:], s_col[:], s_col[:])

            d_sq = small_pool.tile([1, O], F32, tag="dsq")
            d_row = small_pool.tile([1, O], F32, tag="drow")
            d_col = small_pool.tile([128, OT], F32, tag="dcol")

            # demod accumulator lives in one PSUM bank across all 4 c-passes
            ps_d = vpsum_pool.tile([1, O], F32, name="ps_d")

            # ---- weights: all 36 [128, O] bf16 tiles resident in SBUF.
            # Demod partials per c-tile: ps_d[o] += s2[c] * sum_k wt[k][c,o]^2
            wts = [[None] * KK for _ in range(CT)]
            for ct in range(CT):
                c0 = ct * 128
                for k in range(KK):
                    w = w_pool.tile([128, O], BF16, tag=f"w_{ct}_{k}")
                    nc.sync.dma_start(w[:], wt_d[k, c0:c0 + 128, :])
                    wts[ct][k] = w

                w2s = wt2_pool.tile([128, O], F32R, name="w2s", bufs=2)
                nc.scalar.activation(w2s[:], wts[ct][0][:], AF.Square,
                                     bias=zero_col[:, 0:1])
                for k in range(1, KK):
                    w2 = wt2_pool.tile([128, O], F32R, name="w2")
                    nc.scalar.activation(w2[:], wts[ct][k][:], AF.Square,
                                         bias=zero_col[:, 0:1])
                    nc.gpsimd.tensor_add(w2s[:], w2s[:], w2[:])
                nc.tensor.matmul(
                    ps_d[:], s2_col[:, ct:ct + 1], w2s[:],
                    start=(ct == 0), stop=(ct == CT - 1),
                )

            # d = 1/sqrt(ps_d + eps), transposed to columns.
            nc.scalar.activation(d_sq[:], ps_d[:], AF.Sqrt,
                                 bias=eps_1[0:1, 0:1])
            nc.vector.reciprocal(d_row[:], d_sq[:])
            for t in range(OT):
                ps = vpsum_pool.tile([128, 1], F32, name="ps")
                nc.tensor.matmul(
                    ps[:], d_row[0:1, t * 128:(t + 1) * 128],
                    ones[0:1, 0:1], start=True, stop=True,
                )
                nc.scalar.copy(d_col[:, t:t + 1], ps[:])

            # ---- x: padded, s-scaled bf16 image tiles for all 4 c-tiles.
            xps = []
            for ct in range(CT):
                xp = xpad_pool.tile([128, PR, PW], BF16, tag=f"xp_{ct}")
                nc.gpsimd.memset(xp[:].bitcast(mybir.dt.uint16), 0.0)
                xps.append(xp)
            # Row blocks arrive breadth-first across c-tiles so conv chunks
            # unblock in order.
            QR = 8
            for r0 in range(0, H, QR):
                for ct in range(CT):
                    c0 = ct * 128
                    xs = xstg_pool.tile([128, QR, W], F32, name="xs")
                    nc.sync.dma_start(xs[:], x_d[c0:c0 + 128, r0:r0 + QR, :])
                    nc.vector.tensor_scalar_mul(
                        xps[ct][:, 1 + r0:1 + r0 + QR, 1:PW - 1], xs[:],
                        s_col[:, ct:ct + 1])

            # ---- conv: 36 matmuls (9 taps x 4 c-tiles) per output chunk,
            # single PSUM accumulation group, one demod-scaled eviction.
            for ch in range(NCH):
                h0 = ch * ROWS
                for ot in range(OT):
                    o0 = ot * 128
                    ps = cpsum_pool.tile([128, ROWS, W], F32,
                                         name="convps", tag="convps")
                    for ct in range(CT):
                        for k in range(KK):
                            kh, kw = divmod(k, 3)
                            nc.tensor.matmul(
                                ps[:], wts[ct][k][:, o0:o0 + 128],
                                xps[ct][:, h0 + kh:h0 + kh + ROWS,
                                         kw:kw + W],
                                start=(ct == 0 and k == 0),
                                stop=(ct == CT - 1 and k == KK - 1),
                            )
                    osb = osb_pool.tile([128, ROWS, W], F32)
                    nc.scalar.mul(osb[:], ps[:], mul=d_col[:, ot:ot + 1])
                    nc.sync.dma_start(
                        out_d[o0:o0 + 128, h0:h0 + ROWS, :], osb[:])

    nc.compile()
    return nc


def prep_in_maps(x, y, weight):
    """Full inputs -> per-core in_maps (host-side layout prep only)."""
    import ml_dtypes

    x = np.ascontiguousarray(np.asarray(x, dtype=np.float32))
    y = np.ascontiguousarray(np.asarray(y, dtype=np.float32))
    weight = np.asarray(weight, dtype=np.float32)
    # [O, C, 3, 3] -> [9, C, O] bf16 so lhsT tiles ([c, o] per tap) DMA
    # naturally and stream at full PE rate.
    wt = np.ascontiguousarray(
        weight.transpose(2, 3, 1, 0).reshape(KK, C, O).astype(
            ml_dtypes.bfloat16))
    return [{"x": x[b], "y": y[b:b + 1], "wt": wt} for b in range(B)]


_CACHE = {}


def _get_nc():
    if "nc" not in _CACHE:
        _CACHE["nc"] = build_nc()
    return _CACHE["nc"]


def kernel(x, y, weight):
    nc = _get_nc()
    in_maps = prep_in_maps(x, y, weight)
    res = run_bass_kernel_spmd(nc, in_maps, core_ids=list(range(B)))
    kernel.last_results = res
    return np.stack([r["out"] for r in res.results], axis=0)


kernel.last_results = None


# revision 9
# speedup vs baseline: 1.2187x; 1.2187x over previous
"""Conv2DMod (StyleGAN2-style modulated conv) Trainium2 Bass kernel.

Problem: B=8, C_in=512, C_out=512, K=3x3, H=W=64, fp32, 'same' padding.

  wts[b,o,c,kh,kw] = weight[o,c,kh,kw] * (y[b,c]+1)
  d[b,o]           = rsqrt(sum_{c,kh,kw} wts^2 + 1e-8)
  out[b]           = conv2d(x[b], wts[b]*d[b,o])

Strategy (data-parallel over batch, one sample per NeuronCore, 8 cores):
  conv is linear in the weight; the modulation scale s_c=(y+1) depends
  only on the input channel and the demod d_o only on the output
  channel, so
      out = d_o * conv(s_c * x, weight)
  Each core:
    1. preloads ALL weights as bf16 tiles (host pre-transposed to
       [k, c, o]) on the ACT hwdge queue while x streams on the SP
       queue; x is scaled by s_c into 4 H-padded bf16 c-tiles
       [128, 66, 64] (contiguous row writes — W 'same' padding is
       done by narrower shifted matmuls instead of padded columns),
    2. runs the 3x3 conv as 9 shift-matmuls per c-tile, accumulating
       ALL 36 matmuls (9 taps x 4 c-tiles) of an output chunk in a
       single PSUM bank group — one eviction per chunk,
    3. computes d_o on the PE (sum_c s2[c]*w2[c,o] via M=1 matmuls of
       squared weight tiles) and applies it on the PSUM->SBUF eviction.

bf16 operands keep the PE at full streaming rate (same as fp32r on
TRN2), halve weight DMA, and need no on-device fp32r conversion pass;
PSUM accumulation stays fp32 so the rel err is ~2e-3.

kernel(x, y, weight) takes the FULL unsharded inputs and returns the
full (8, 512, 64, 64) fp32 output.
"""

import numpy as np

import concourse.bass as bass
import concourse.tile as tile
from concourse import bacc, mybir
from concourse.bass_utils import run_bass_kernel_spmd

# Problem constants (hardcoded per spec).
B = 8
C = 512          # input channels
O = 512          # output channels
H = W = 64
KK = 9           # 3x3 taps
PRH = 66         # H-padded rows
CT = 4           # c tiles of 128
OT = 4           # o tiles of 128
NCH = 8          # hw chunks: 8 rows x 64 cols = 512 free elems
ROWS = 8
EPS = 1e-8

F32 = mybir.dt.float32
F32R = mybir.dt.float32r
BF16 = mybir.dt.bfloat16
AF = mybir.ActivationFunctionType

# tap order: full-width (kw=1) taps first so the group's first matmul
# (start=True, clears the PSUM bank) covers every element of the bank.
TAPS = [1, 4, 7, 0, 3, 6, 2, 5, 8]


def build_nc(reps=1):
    nc = bacc.Bacc(None, target_bir_lowering=False)

    x_d = nc.dram_tensor("x", [C, H, W], F32, kind="ExternalInput")
    y_d = nc.dram_tensor("y", [1, C], F32, kind="ExternalInput")
    wt_d = nc.dram_tensor("wt", [KK, C, O], BF16, kind="ExternalInput")
    out_d = nc.dram_tensor("out", [O, H, W], F32, kind="ExternalOutput")

    with tile.TileContext(nc) as tc:
      for _rep in range(reps):
        with (
            tc.tile_pool(name="wsb", bufs=1) as w_pool,
            tc.tile_pool(name="xpad", bufs=1) as xpad_pool,
            tc.tile_pool(name="xstg", bufs=8) as xstg_pool,
            tc.tile_pool(name="wt2", bufs=3) as wt2_pool,
            tc.tile_pool(name="osb", bufs=4) as osb_pool,
            tc.tile_pool(name="small", bufs=1) as small_pool,
            tc.tile_pool(name="cpsum", bufs=6, space=bass.MemorySpace.PSUM) as cpsum_pool,
            tc.tile_pool(name="vpsum", bufs=1, space=bass.MemorySpace.PSUM) as vpsum_pool,
        ):
            # ---- small setup: s = y+1, transposed to per-partition cols
            y_sb = small_pool.tile([1, C], F32, tag="y")
            nc.sync.dma_start(y_sb[:], y_d[:])

            ones = small_pool.tile([1, 1], F32, tag="ones")
            nc.vector.memset(ones[:], 1.0)
            eps_1 = small_pool.tile([1, 1], F32, tag="eps1")
            nc.vector.memset(eps_1[:], EPS)
            zero_col = small_pool.tile([128, 1], F32, tag="zerocol")
            nc.vector.memset(zero_col[:], 0.0)

            s_row = small_pool.tile([1, C], F32, tag="srow")
            nc.scalar.activation(s_row[:], y_sb[:], AF.Identity,
                                 bias=ones[0:1, 0:1])

            # K=1 matmul transposes a row-vector slice into a psum column.
            s_col = small_pool.tile([128, CT], F32, tag="scol")
            for t in range(CT):
                ps = vpsum_pool.tile([128, 1], F32, name="ps")
                nc.tensor.matmul(
                    ps[:], s_row[0:1, t * 128:(t + 1) * 128], ones[0:1, 0:1],
                    start=True, stop=True,
                )
                nc.scalar.copy(s_col[:, t:t + 1], ps[:])
            # fp32r matmul operands must be engine-produced with F32R
            # output dtype.
            s2_col = small_pool.tile([128, CT], F32R, tag="s2col")
            nc.vector.tensor_mul(s2_col[:], s_col[:], s_col[:])

            d_sq = small_pool.tile([1, O], F32, tag="dsq")
            d_row = small_pool.tile([1, O], F32, tag="drow")
            d_col = small_pool.tile([128, OT], F32, tag="dcol")

            # demod accumulator lives in one PSUM bank across all 4 c-passes
            ps_d = vpsum_pool.tile([1, O], F32, name="ps_d")

            # ---- weights on the ACT hwdge queue (x uses SP): all 36
            # [128, O] bf16 tiles resident in SBUF.  Demod partials per
            # c-tile: ps_d[o] += s2[c] * sum_k wt[k][c,o]^2
            wts = [[None] * KK for _ in range(CT)]
            for ct in range(CT):
                c0 = ct * 128
                for k in range(KK):
                    w = w_pool.tile([128, O], BF16, tag=f"w_{ct}_{k}")
                    nc.scalar.dma_start(w[:], wt_d[k, c0:c0 + 128, :])
                    wts[ct][k] = w

                w2s = wt2_pool.tile([128, O], F32R, name="w2s", bufs=2)
                nc.scalar.activation(w2s[:], wts[ct][0][:], AF.Square,
                                     bias=zero_col[:, 0:1])
                for k in range(1, KK):
                    w2 = wt2_pool.tile([128, O], F32R, name="w2")
                    nc.scalar.activation(w2[:], wts[ct][k][:], AF.Square,
                                         bias=zero_col[:, 0:1])
                    nc.gpsimd.tensor_add(w2s[:], w2s[:], w2[:])
                nc.tensor.matmul(
                    ps_d[:], s2_col[:, ct:ct + 1], w2s[:],
                    start=(ct == 0), stop=(ct == CT - 1),
                )

            # d = 1/sqrt(ps_d + eps), transposed to columns.
            nc.scalar.activation(d_sq[:], ps_d[:], AF.Sqrt,
                                 bias=eps_1[0:1, 0:1])
            nc.vector.reciprocal(d_row[:], d_sq[:])
            for t in range(OT):
                ps = vpsum_pool.tile([128, 1], F32, name="ps")
                nc.tensor.matmul(
                    ps[:], d_row[0:1, t * 128:(t + 1) * 128],
                    ones[0:1, 0:1], start=True, stop=True,
                )
                nc.scalar.copy(d_col[:, t:t + 1], ps[:])

            # ---- x: H-padded, s-scaled bf16 image tiles, one per c-tile.
            # Rows are contiguous 64-col segments (no W padding), so the
            # DVE scale writes are contiguous.
            xps = []
            for ct in range(CT):
                xp = xpad_pool.tile([128, PRH, W], BF16, tag=f"xp_{ct}")
                nc.gpsimd.memset(xp[:, 0:1, :].bitcast(mybir.dt.uint16), 0.0)
                nc.gpsimd.memset(
                    xp[:, PRH - 1:PRH, :].bitcast(mybir.dt.uint16), 0.0)
                xps.append(xp)
            # Row blocks arrive breadth-first across c-tiles so conv chunks
            # unblock in order.
            QR = 8
            for r0 in range(0, H, QR):
                for ct in range(CT):
                    c0 = ct * 128
                    xs = xstg_pool.tile([128, QR, W], F32, name="xs")
                    nc.sync.dma_start(xs[:], x_d[c0:c0 + 128, r0:r0 + QR, :])
                    nc.vector.tensor_scalar_mul(
                        xps[ct][:, 1 + r0:1 + r0 + QR, :], xs[:],
                        s_col[:, ct:ct + 1])

            # ---- conv: 36 matmuls (9 taps x 4 c-tiles) per output chunk,
            # single PSUM accumulation group, one demod-scaled eviction.
            # W 'same' padding via narrower shifted matmuls:
            #   kw=0 (left tap):  out cols 1..63 <- in cols 0..62
            #   kw=1 (center):    out cols 0..63 <- in cols 0..63
            #   kw=2 (right tap): out cols 0..62 <- in cols 1..63
            for ch in range(NCH):
                h0 = ch * ROWS
                for ot in range(OT):
                    o0 = ot * 128
                    ps = cpsum_pool.tile([128, ROWS, W], F32,
                                         name="convps", tag="convps")
                    first = True
                    for ki, k in enumerate(TAPS):
                        kh, kw = divmod(k, 3)
                        for ct in range(CT):
                            rows = xps[ct][:, h0 + kh:h0 + kh + ROWS, :]
                            if kw == 0:
                                rhs = rows[:, :, 0:W - 1]
                                dst = ps[:, :, 1:W]
                            elif kw == 1:
                                rhs = rows
                                dst = ps[:]
                            else:
                                rhs = rows[:, :, 1:W]
                                dst = ps[:, :, 0:W - 1]
                            nc.tensor.matmul(
                                dst, wts[ct][k][:, o0:o0 + 128], rhs,
                                start=first,
                                stop=(ki == KK - 1 and ct == CT - 1),
                                skip_group_check=True,
                            )
                            first = False
                    osb = osb_pool.tile([128, ROWS, W], F32)
                    nc.scalar.mul(osb[:], ps[:], mul=d_col[:, ot:ot + 1])
                    nc.scalar.dma_start(
                        out_d[o0:o0 + 128, h0:h0 + ROWS, :], osb[:])

    nc.compile()
    return nc


def prep_in_maps(x, y, weight):
    """Full inputs -> per-core in_maps (host-side layout prep only)."""
    import ml_dtypes

    x = np.ascontiguousarray(np.asarray(x, dtype=np.float32))
    y = np.ascontiguousarray(np.asarray(y, dtype=np.float32))
    weight = np.asarray(weight, dtype=np.float32)
    # [O, C, 3, 3] -> [9, C, O] bf16 so lhsT tiles ([c, o] per tap) DMA
    # naturally and stream at full PE rate.
    wt = np.ascontiguousarray(
        weight.transpose(2, 3, 1, 0).reshape(KK, C, O).astype(
            ml_dtypes.bfloat16))
    return [{"x": x[b], "y": y[b:b + 1], "wt": wt} for b in range(B)]


_CACHE = {}


def _get_nc():
    if "nc" not in _CACHE:
        _CACHE["nc"] = build_nc()
    return _CACHE["nc"]


def kernel(x, y, weight):
    nc = _get_nc()
    in_maps = prep_in_maps(x, y, weight)
    res = run_bass_kernel_spmd(nc, in_maps, core_ids=list(range(B)))
    kernel.last_results = res
    return np.stack([r["out"] for r in res.results], axis=0)


kernel.last_results = None
